# revision 24
# baseline (speedup 1.0000x reference)
"""Trainium2 Bass kernel for AssociativeMemoryModule (causal linear attention).

Sharding: head-parallel with output-partial unshard - core c owns head c for
both batches and NEVER communicates on-device (no collectives, no NRT
pre-collective barrier). Each core:
  1. projects full x (pre-transposed, bf16 on host) to [q.T;k.T] (128 rows)
     and v.T (64 rows); phi = min(exp(z),1) + relu(z) in f32 -> bf16,
  2. stacks [kT; vT] at base partition 0 so one PE transpose per 128-chunk
     yields both k and v in normal layout; masked scores batched per t-tile,
  3. chunked causal linear attention (C=128): kv outer products + DVE prefix
     adds reading PSUM directly; po epilogue in waves of 4,
  4. o-projection PARTIAL for its own head only: per chunk,
     o_part[t,:] = onT_chunk.T @ Wo_head.T (512-col matmul), written out as
     bf16 per chunk.
Host unshards by summing the 8 per-core partials over heads and adding bo.
"""
import sys

import numpy as np

sys.path.insert(0, "/opt/trn_rl_repo")

H, HD, D = 8, 64, 512
B, T = 2, 1024
BT = B * T            # 2048
C = 128               # attention chunk
NCH = BT // C         # 16 chunks total
CPB = T // C          # 8 chunks per batch
NF = D // 128         # 4 feature tiles
NT = 4                # t-tiles of 512 for projections

_CACHE = {}


def _build():
    if "nc" in _CACHE:
        return _CACHE["nc"]
    import concourse.mybir as mybir
    import concourse.tile as tile
    from concourse import bacc
    from concourse.bass import ts

    import ml_dtypes

    f32 = mybir.dt.float32
    bf16 = mybir.dt.bfloat16
    AF = mybir.ActivationFunctionType

    nc = bacc.Bacc("TRN2", target_bir_lowering=False, debug=False, num_devices=8,
                   num_swdge_queues=4)

    xT = nc.declare_dram_parameter("xT", [D, BT], bf16, isOutput=False)
    wa = nc.declare_dram_parameter("wa", [D, 128], bf16, isOutput=False)
    wv = nc.declare_dram_parameter("wv", [D, HD], bf16, isOutput=False)
    woh = nc.declare_dram_parameter("woh", [HD, D], bf16, isOutput=False)
    bqk = nc.declare_dram_parameter("bqk", [128, 1], f32, isOutput=False)
    bv = nc.declare_dram_parameter("bv", [HD, 1], f32, isOutput=False)
    out = nc.declare_dram_parameter("out", [NCH, C, D], bf16, isOutput=True)

    # mask4[s, jj, t] = s <= t (same causal mask for each of 4 chunks)
    mask_np = np.broadcast_to(
        np.triu(np.ones((C, C), np.float32))[:, None, :], (C, 4, C)).copy()
    iden128_np = np.eye(C, dtype=ml_dtypes.bfloat16)
    mask_d = nc.inline_tensor(mask_np.reshape(C, 4 * C), "causal_mask4")
    iden128_d = nc.inline_tensor(iden128_np, "iden128")

    with tile.TileContext(nc) as tc:
        with (
            tc.tile_pool(name="consts", bufs=1) as consts,
        ):
            # ---- resident SBUF tensors (matmul operands in bf16) ----
            xt_sb = consts.tile([128, NF, BT], bf16)
            wa_sb = consts.tile([128, NF, 128], bf16)
            wv_sb = consts.tile([128, NF, HD], bf16)
            woh_sb = consts.tile([HD, D], bf16)
            bqk_sb = consts.tile([128, 1], f32)
            bv_sb = consts.tile([HD, 1], f32)
            mask_sb = consts.tile([C, 4, C], f32)
            iden128_sb = consts.tile([C, C], bf16)
            qk_phi = consts.tile([128, BT], bf16)      # rows 0-63 qT, 64-127 kT
            kvT = consts.tile([128, BT], bf16)         # rows 0-63 kT, 64-127 vT
            vT_sb = consts.tile([HD, BT], bf16)
            k_nrm = consts.tile([128, NCH, HD], bf16)
            v_aug = consts.tile([128, NCH, HD + 1], bf16)
            sm_all = consts.tile([C, NCH, C], bf16)
            Sf = consts.tile([HD, B, CPB - 1, HD + 1], f32)
            Sb16 = consts.tile([HD, B, CPB - 1, HD + 1], bf16)

            # ---- input staging. Each dma_start blocks its queue ~600ns, so
            # the per-queue issue ORDER is the schedule: first-needed first.
            def xt_dma(eng, tcol, f):
                eng.dma_start(xt_sb[:, f, ts(tcol, 512)],
                              xT[128 * f:128 * (f + 1), ts(tcol, 512)])

            nc.scalar.dma_start(wa_sb[:, 0, :], wa[0:128, :])
            nc.scalar.dma_start(wa_sb[:, 1, :], wa[128:256, :])
            xt_dma(nc.sync, 0, 0)
            xt_dma(nc.gpsimd, 0, 1)
            nc.scalar.dma_start(wa_sb[:, 2, :], wa[256:384, :])
            nc.scalar.dma_start(wa_sb[:, 3, :], wa[384:512, :])
            xt_dma(nc.sync, 0, 2)
            xt_dma(nc.scalar, 0, 3)
            nc.scalar.dma_start(bqk_sb[:], bqk[:, :])
            nc.scalar.dma_start(bv_sb[:], bv[:, :])
            for f in range(NF):
                nc.gpsimd.dma_start(wv_sb[:, f, :], wv[128 * f:128 * (f + 1), :])
            nc.gpsimd.dma_start(iden128_sb[:], iden128_d[:, :])
            nc.gpsimd.dma_start(mask_sb[:],
                                mask_d.ap().rearrange("p (j t) -> p j t", j=4))
            nc.vector.memset(v_aug[:, :, HD:HD + 1], 1.0)
            for tcol in range(1, NT):
                xt_dma(nc.sync, tcol, 0)
                xt_dma(nc.scalar, tcol, 1)
                xt_dma(nc.sync, tcol, 2)
                xt_dma(nc.scalar, tcol, 3)
            nc.sync.dma_start(woh_sb[:], woh[:, :])

            with (
                tc.tile_pool(name="psA", bufs=2, space="PSUM") as psA,
                tc.tile_pool(name="psB", bufs=1, space="PSUM") as psB,
                tc.tile_pool(name="psT", bufs=2, space="PSUM") as psT,
                tc.tile_pool(name="psSc", bufs=1, space="PSUM") as psSc,
                tc.tile_pool(name="psO", bufs=2, space="PSUM") as psO,
                tc.tile_pool(name="ptmp", bufs=2) as ptmp,
                tc.tile_pool(name="attn", bufs=4) as attn,
            ):
                def proj_tile(tt):
                    sl = ts(tt, 512)
                    pa = psA.tile([128, 512], f32, tag="pa", name=f"pa{tt}")
                    pb = psB.tile([HD, 512], f32, tag="pb", name=f"pb{tt}")
                    for f in range(NF):
                        nc.tensor.matmul(pa, wa_sb[:, f, :], xt_sb[:, f, sl],
                                         start=(f == 0), stop=(f == NF - 1))
                    for f in range(NF):
                        nc.tensor.matmul(pb, wv_sb[:, f, :], xt_sb[:, f, sl],
                                         start=(f == 0), stop=(f == NF - 1))
                    nc.scalar.activation(vT_sb[:, sl], pb, AF.Identity, bias=bv_sb[:])
                    # phi = exp(min(z,0)) + relu(z) = min(exp(z),1) + relu(z):
                    # both ACT ops read PSUM directly with fused bias
                    rr = ptmp.tile([128, 512], f32, tag="rr", name=f"rr{tt}")
                    ee = ptmp.tile([128, 512], f32, tag="ee", name=f"ee{tt}")
                    mm = ptmp.tile([128, 512], f32, tag="mm", name=f"mm{tt}")
                    nc.scalar.activation(ee, pa, AF.Exp, bias=bqk_sb[:])
                    nc.scalar.activation(rr, pa, AF.Relu, bias=bqk_sb[:])
                    nc.vector.tensor_scalar_min(mm, ee, 1.0)
                    nc.vector.tensor_add(qk_phi[:, sl], mm, rr)
                    # build [kT; vT] at base partition 0 (SBUF->SBUF DMAs):
                    # matmul operands must share a base partition, and the
                    # stacked tile transposes k and v chunks in ONE PE op
                    nc.sync.dma_start(kvT[0:HD, sl], qk_phi[64:128, sl])
                    nc.gpsimd.dma_start(kvT[HD:128, sl], vT_sb[:, sl])
                    # transposes + scores for the 4 chunks in this t-tile
                    ptr = psT.tile([C, 4, C], bf16, tag="tr", name=f"tr{tt}")
                    psc = psSc.tile([C, 4, C], f32, tag="ps", name=f"ps{tt}")
                    for jj in range(4):
                        i = tt * 4 + jj
                        cs = ts(i, C)
                        nc.tensor.transpose(ptr[:, jj, :], kvT[:, cs],
                                            iden128_sb[:])
                        nc.tensor.matmul(psc[:, jj, :], kvT[0:HD, cs],
                                         qk_phi[0:64, cs], start=True, stop=True)
                    i0 = tt * 4
                    nc.scalar.copy(k_nrm[:, i0:i0 + 4, :], ptr[:, :, 0:HD])
                    nc.vector.tensor_copy(v_aug[:, i0:i0 + 4, 0:HD],
                                          ptr[:, :, HD:2 * HD])
                    nc.vector.tensor_mul(sm_all[:, i0:i0 + 4, :], psc, mask_sb[:])

                def kv_part(b):
                    # kv products + incremental prefix state (j = 0..CPB-2)
                    for w in range(2):
                        pkv = psO.tile([HD, 4, HD + 1], f32, tag="po",
                                       name=f"pkv{b}{w}")
                        for jw in range(4 if w == 0 else 3):
                            j = 4 * w + jw
                            i = b * CPB + j
                            nc.tensor.matmul(pkv[:, jw, :], k_nrm[:, i, :],
                                             v_aug[:, i, :], start=True, stop=True)
                        for jw in range(4 if w == 0 else 3):
                            j = 4 * w + jw
                            if j == 0:
                                nc.vector.tensor_copy(Sf[:, b, 0, :], pkv[:, 0, :])
                            else:
                                nc.vector.tensor_add(Sf[:, b, j, :],
                                                     Sf[:, b, j - 1, :],
                                                     pkv[:, jw, :])
                            nc.scalar.copy(Sb16[:, b, j, :], Sf[:, b, j, :])

                def po_part(b):
                    # waves of 4 chunks: po matmuls, DVE epilogue, transposes,
                    # SBUF bounce, then the per-chunk o-projection PARTIAL
                    # (own head only) streamed straight to DRAM
                    for w in range(2):
                        po = psO.tile([C, 4, HD + 1], f32, tag="po",
                                      name=f"po{b}{w}")
                        ptr = psT.tile([HD, 4, C], bf16, tag="tr",
                                       name=f"otr{b}{w}")
                        for jw in range(4):
                            j = 4 * w + jw
                            i = b * CPB + j
                            cs = ts(i, C)
                            if j == 0:
                                nc.tensor.matmul(po[:, jw, :], sm_all[:, i, :],
                                                 v_aug[:, i, :],
                                                 start=True, stop=True)
                            else:
                                nc.tensor.matmul(po[:, jw, :], sm_all[:, i, :],
                                                 v_aug[:, i, :],
                                                 start=True, stop=False)
                                nc.tensor.matmul(po[:, jw, :], qk_phi[0:64, cs],
                                                 Sb16[:, b, j - 1, :],
                                                 start=False, stop=True)
                        for jw in range(4):
                            j = 4 * w + jw
                            i = b * CPB + j
                            # denom > 0 always (phi > 0); reference's 1e-6
                            # clamp can never bind at these magnitudes
                            dr = attn.tile([C, 1], f32, tag="dr", name=f"dr{i}")
                            nc.vector.reciprocal(dr, po[:, jw, HD:HD + 1])
                            on = attn.tile([C, HD], bf16, tag="on", name=f"on{i}")
                            nc.vector.tensor_scalar_mul(on, po[:, jw, 0:HD], dr)
                            nc.tensor.transpose(ptr[:, jw, :], on, iden128_sb[:])
                        ot = attn.tile([HD, 4, C], bf16, tag="ot", name=f"ot{b}{w}")
                        nc.scalar.copy(ot[:], ptr[:])
                        for jw in range(4):
                            j = 4 * w + jw
                            i = b * CPB + j
                            pp = psA.tile([C, D], f32, tag="pa", name=f"pp{i}")
                            nc.tensor.matmul(pp, ot[:, jw, :], woh_sb[:],
                                             start=True, stop=True)
                            osl = attn.tile([C, D], bf16, tag="osl",
                                            name=f"osl{i}")
                            nc.vector.tensor_copy(osl, pp)
                            eng = (nc.sync, nc.gpsimd)[i % 2]
                            eng.dma_start(out[i, :, :], osl)

                proj_tile(0)
                proj_tile(1)
                kv_part(0)
                po_part(0)
                proj_tile(2)
                proj_tile(3)
                kv_part(1)
                po_part(1)

    nc.compile()
    _CACHE["nc"] = nc
    return nc


def _in_maps(x, Wq, bq, Wk, bk, Wv, bv, Wo, bo):
    import ml_dtypes
    bf = ml_dtypes.bfloat16
    x2 = np.ascontiguousarray(x.reshape(BT, D).T).astype(bf)
    WoT = np.ascontiguousarray(Wo.T)                  # [(h m), d]
    maps = []
    for c in range(8):
        sl = slice(HD * c, HD * (c + 1))
        maps.append(dict(
            xT=x2,
            wa=np.ascontiguousarray(np.concatenate([Wq[sl], Wk[sl]], 0).T).astype(bf),
            wv=np.ascontiguousarray(Wv[sl].T).astype(bf),
            woh=np.ascontiguousarray(WoT[sl]).astype(bf),
            bqk=np.ascontiguousarray(np.concatenate([bq[sl], bk[sl]]).reshape(128, 1)).astype(np.float32),
            bv=np.ascontiguousarray(bv[sl].reshape(HD, 1)).astype(np.float32),
        ))
    return maps


def kernel(x, Wq, bq, Wk, bk, Wv, bv, Wo, bo):
    from concourse import bass_utils

    nc = _build()
    maps = _in_maps(np.asarray(x), np.asarray(Wq), np.asarray(bq),
                    np.asarray(Wk), np.asarray(bk), np.asarray(Wv),
                    np.asarray(bv), np.asarray(Wo), np.asarray(bo))
    res = bass_utils.run_bass_kernel_spmd(nc, maps, core_ids=list(range(8)))
    # unshard: heads were output-partial sharded -> sum partials, add bias
    acc = np.zeros((NCH, C, D), np.float32)
    for c in range(8):
        acc += res.results[c]["out"].astype(np.float32)
    o = acc.reshape(B, T, D) + np.asarray(bo).astype(np.float32)[None, None, :]
    return np.ascontiguousarray(o).astype(np.float32)
